# revision 7
# baseline (speedup 1.0000x reference)
"""Multi-head causal attention (B=4, T=2048, K=1024, H=16) on 8 NeuronCores.

Sharding: data parallel over B (4) x tensor parallel over heads (2 groups of 8).
Each core computes QKV projections for its 8 heads, causal attention, and a
partial output projection (Wp row-partitioned); the host sums the two partial
yT tensors per batch and adds the bias.

All matmuls run in float32r (TF32-like, ~1e-4 rel err) at full PE rate.
Scores are built directly in transposed layout (P~T[u,t] = exp(kT.T @ qT / 4))
so the PV matmul needs no on-chip transposes; a ones-column appended to V
yields the softmax denominator from the same matmul.
"""
import sys
sys.path.insert(0, '/opt/trn_rl_repo')
import numpy as np

B, T, K, H = 4, 2048, 1024, 16
S = K // H          # 64 head dim
G = 2               # head groups (tensor parallel)
HG = H // G         # 8 heads per core
F = K // G          # 512 features per core
NCORES = 8
NF = K // 128       # 8 contraction chunks
NMB = F // 128      # 4 feature blocks per core
NTB = T // 512      # 4 t-blocks of 512
NU = T // 128       # 16 u-chunks of 128
SCALE = float(H) ** -0.5  # 0.25

_CACHE = {}


def _build():
    import concourse.tile as tile
    import concourse.mybir as mybir
    from concourse import bacc

    dt = mybir.dt
    F32 = dt.float32
    F32R = dt.float32r
    AF = mybir.ActivationFunctionType
    MUL = mybir.AluOpType.mult

    nc = bacc.Bacc("TRN2", target_bir_lowering=False, debug=False)

    x_d = nc.dram_tensor("x_l", [128, NF, T], F32R, kind="ExternalInput")
    wq_d = nc.dram_tensor("wq_l", [128, NF, F], F32R, kind="ExternalInput")
    wk_d = nc.dram_tensor("wk_l", [128, NF, F], F32R, kind="ExternalInput")
    wv_d = nc.dram_tensor("wv_l", [128, NF, F], F32R, kind="ExternalInput")
    wp_d = nc.dram_tensor("wp_l", [128, NMB, K], F32R, kind="ExternalInput")
    msk_d = nc.dram_tensor("msk", [128, 4, 512], F32R, kind="ExternalInput")
    y_d = nc.dram_tensor("yT", [K, T], F32, kind="ExternalOutput")

    with tile.TileContext(nc) as tc:
        with tc.tile_pool(name="persist", bufs=1) as pp:

            qT = pp.tile([128, NMB, T], F32R)        # 4 MB
            kT = pp.tile([128, NMB, T], F32R)        # 4 MB
            v_sb = pp.tile([128, NU, HG, S + 1], F32R)  # 4.26 MB
            outAT = pp.tile([128, NMB, T], F32R)     # 4 MB
            ones_r = pp.tile([1, S], F32R)

            # ---------------- Phase 1: QKV projections ----------------
            with tc.tile_pool(name="wqkv", bufs=1) as wqkv_pool, \
                 tc.tile_pool(name="xs", bufs=2) as xs_pool, \
                 tc.tile_pool(name="ps_acc", bufs=5, space="PSUM") as ps_acc, \
                 tc.tile_pool(name="ps_v", bufs=2, space="PSUM") as ps_v:
                wq = wqkv_pool.tile([128, NF, F], F32R, tag="wq")  # 2 MB
                wk = wqkv_pool.tile([128, NF, F], F32R, tag="wk")
                wv = wqkv_pool.tile([128, NF, F], F32R, tag="wv")
                nc.sync.dma_start(wq[:], wq_d[:])
                nc.sync.dma_start(wk[:], wk_d[:])
                nc.sync.dma_start(wv[:], wv_d[:])

                ones_f = wqkv_pool.tile([1, S], F32, tag="ones_f")
                nc.vector.memset(ones_f[:], 1.0)
                nc.vector.tensor_copy(ones_r[:], ones_f[:])
                vcol_f = wqkv_pool.tile([128, NU * HG], F32, tag="vcol")
                nc.vector.memset(vcol_f[:], 1.0)
                nc.vector.tensor_copy(
                    v_sb[:, :, :, S:S + 1],
                    vcol_f[:].rearrange("p (a b) -> p a b", a=NU)[:, :, :, None],
                )

                for tb in range(T // 256):  # 256-token blocks
                    x_tb = xs_pool.tile([128, NF, 256], F32R, tag="x")
                    nc.sync.dma_start(x_tb[:], x_d[:, :, tb * 256:(tb + 1) * 256])
                    for w_sb, dst in ((wq, qT), (wk, kT)):
                        for mb in range(NMB):
                            acc = ps_acc.tile([128, 256], F32, tag="acc")
                            for f in range(NF):
                                nc.tensor.matmul(
                                    acc[:],
                                    w_sb[:, f, mb * 128:(mb + 1) * 128],
                                    x_tb[:, f, :],
                                    start=(f == 0), stop=(f == NF - 1),
                                )
                            nc.vector.tensor_copy(
                                dst[:, mb, tb * 256:(tb + 1) * 256], acc[:])
                    for tt in range(2):
                        ub = tb * 2 + tt
                        accv = ps_v.tile([128, 512], F32, tag="v")
                        for f in range(NF):
                            nc.tensor.matmul(
                                accv[:],
                                x_tb[:, f, tt * 128:(tt + 1) * 128],
                                wv[:, f, :],
                                start=(f == 0), stop=(f == NF - 1),
                            )
                        nc.vector.tensor_copy(
                            v_sb[:, ub, :, 0:S],
                            accv[:].rearrange("p (h s) -> p h s", h=HG),
                        )

            # ---------------- Phase 2: causal attention ----------------
            with tc.tile_pool(name="wp_pool", bufs=1) as wp_pool, \
                 tc.tile_pool(name="pexp", bufs=8) as pexp, \
                 tc.tile_pool(name="small", bufs=2) as sm, \
                 tc.tile_pool(name="ysb", bufs=4) as ysb_pool:
              wp_sb = wp_pool.tile([128, NMB, K], F32R, tag="wp")  # 2 MB
              nc.sync.dma_start(wp_sb[:], wp_d[:])
              msk = wp_pool.tile([128, 4, 512], F32R, tag="msk")  # 1 MB
              nc.sync.dma_start(msk[:], msk_d[:])

              with tc.tile_pool(name="ps_sc", bufs=3, space="PSUM") as ps_sc, \
                 tc.tile_pool(name="ps_pv", bufs=2, space="PSUM") as ps_pv, \
                 tc.tile_pool(name="ps_bc", bufs=2, space="PSUM") as ps_bc:
                for h in range(HG):
                    po = (h & 1) * S
                    mb = h >> 1
                    for tb in range(NTB):
                        nu = 4 * tb + 4
                        pv = ps_pv.tile([S + 1, 512], F32, tag="pv")
                        for ub in range(nu):
                            sc = ps_sc.tile([128, 512], F32, tag="sc")
                            nc.tensor.matmul(
                                sc[:],
                                kT[po:po + S, mb, ub * 128:(ub + 1) * 128],
                                qT[po:po + S, mb, tb * 512:(tb + 1) * 512],
                                start=True, stop=True,
                            )
                            pt = pexp.tile([128, 512], F32R, tag="pt")
                            nc.scalar.activation(pt[:], sc[:], AF.Exp, scale=SCALE)
                            if ub >= 4 * tb:  # diagonal block: apply causal mask
                                ptm = pexp.tile([128, 512], F32R, tag="pt")
                                nc.vector.tensor_tensor(
                                    ptm[:], pt[:], msk[:, ub - 4 * tb, :], MUL)
                                pt = ptm
                            nc.tensor.matmul(
                                pv[:], v_sb[:, ub, h, :], pt[:],
                                start=(ub == 0), stop=(ub == nu - 1),
                            )
                        # normalize: out[s,t] = pv[s,t] / pv[S,t]
                        recip = sm.tile([1, 512], F32, tag="recip")
                        nc.vector.reciprocal(recip[:], pv[S:S + 1, :])
                        recip_r = sm.tile([1, 512], F32R, tag="recip_r")
                        nc.vector.tensor_copy(recip_r[:], recip[:])
                        bc = ps_bc.tile([S, 512], F32, tag="bc")
                        nc.tensor.matmul(bc[:], ones_r[:], recip_r[:],
                                         start=True, stop=True)
                        bc_sb = sm.tile([S, 512], F32, tag="bc_sb")
                        nc.vector.tensor_copy(bc_sb[:], bc[:])
                        nc.vector.tensor_tensor(
                            outAT[po:po + S, mb, tb * 512:(tb + 1) * 512],
                            pv[0:S, :], bc_sb[:], MUL)

              # ---------------- Phase 3: output projection ----------------
              with tc.tile_pool(name="ps_y", bufs=8, space="PSUM") as ps_y:
                for jb in range(K // 128):
                    ytiles = [ps_y.tile([128, 512], F32, tag="y", name=f"yt{jb}_{tb}")
                              for tb in range(NTB)]
                    for i in range(NMB):
                        for tb in range(NTB):
                            nc.tensor.matmul(
                                ytiles[tb],
                                wp_sb[:, i, jb * 128:(jb + 1) * 128],
                                outAT[:, i, tb * 512:(tb + 1) * 512],
                                start=(i == 0), stop=(i == NMB - 1),
                            )
                    for tb in range(NTB):
                        ysb = ysb_pool.tile([128, 512], F32, tag="ysb")
                        nc.vector.tensor_copy(ysb[:], ytiles[tb])
                        nc.sync.dma_start(
                            y_d[jb * 128:(jb + 1) * 128,
                                tb * 512:(tb + 1) * 512],
                            ysb[:])

    nc.compile()
    return nc


def _prep_inputs(input_data, Wq, Wk, Wv, Wp):
    """Build the 8 per-core input maps (all arrays fp32, fed to f32r tensors)."""
    f32 = np.float32
    # causal diagonal masks: msk[ul, j, tl] = 1 if 128*j + ul <= tl else 0
    msk = np.zeros((128, 4, 512), f32)
    ul = np.arange(128)[:, None]
    tl = np.arange(512)[None, :]
    for j in range(4):
        msk[:, j, :] = (128 * j + ul <= tl).astype(f32)

    in_maps = []
    for c in range(NCORES):
        b, g = c // G, c % G
        rows = slice(g * F, (g + 1) * F)
        xT = np.ascontiguousarray(input_data[b].T)              # [K, T]
        x_l = np.ascontiguousarray(
            xT.reshape(NF, 128, T).transpose(1, 0, 2))          # [128, NF, T]
        def wl(W):
            wt = np.ascontiguousarray(W[rows, :].T)             # [K, F]
            return np.ascontiguousarray(
                wt.reshape(NF, 128, F).transpose(1, 0, 2))      # [128, NF, F]
        wpt = np.ascontiguousarray(Wp[:, rows].T)               # [F, K]
        wp_l = np.ascontiguousarray(
            wpt.reshape(NMB, 128, K).transpose(1, 0, 2))        # [128, NMB, K]
        in_maps.append({
            "x_l": x_l, "wq_l": wl(Wq), "wk_l": wl(Wk), "wv_l": wl(Wv),
            "wp_l": wp_l, "msk": msk,
        })
    return in_maps


def kernel(input_data, Wq, Wk, Wv, Wp, bp, _trace=False):
    from concourse.bass_utils import run_bass_kernel_spmd

    if "nc" not in _CACHE:
        _CACHE["nc"] = _build()
    nc = _CACHE["nc"]

    in_maps = _prep_inputs(
        np.asarray(input_data, np.float32), np.asarray(Wq, np.float32),
        np.asarray(Wk, np.float32), np.asarray(Wv, np.float32),
        np.asarray(Wp, np.float32))

    br = run_bass_kernel_spmd(nc, in_maps, core_ids=list(range(NCORES)),
                              trace=_trace)
    _CACHE["last_result"] = br

    bp = np.asarray(bp, np.float32)
    y = np.empty((B, T, K), np.float32)
    for b in range(B):
        acc = br.results[2 * b]["yT"] + br.results[2 * b + 1]["yT"]  # [K, T]
        y[b] = acc.T + bp
    return y
